# revision 15
# baseline (speedup 1.0000x reference)
"""Trainium2 Bass kernel for a transformer decoder layer (self-attn +
cross-attn + FFN), data-parallel over batch across 8 NeuronCores.

Layout strategy per core (2 batch items per core):
  - Activations feeding matmuls are kept feature-major ([C, L] with C on
    partitions) so projections contract over the partition dim; LN /
    residual work is token-major ([L, C]).  Transposes go through the PE
    (bf16, via identity matmul).
  - Attention uses the S^T formulation: S^T[lk, lq] tiles come straight
    out of matmuls with K/Q feature-major; exp on ScalarE; att@V uses V
    token-major (augmented with a ones column so sumexp falls out of the
    same matmul), accumulated in a persistent PSUM tile across chunks.
  - Positional-encoding projections (qpos@Wq etc.) are precomputed on the
    host and folded in as additive terms, so `memory` only needs one
    on-device transpose.
  - All matmul operands are bf16 (fp32 accumulate in PSUM); everything
    else (softmax recip, LN, residuals) is fp32.
  - Emission is software-pipelined: memory-chunk prep work (DMA, cast,
    transpose, K/V projection) for chunk c+2 is interleaved between the
    per-head score/exp/att-V steps of chunk c via a filler queue, so the
    in-order PE queue always has independent matmuls while ScalarE's exp
    and the LN chains run.
"""

import os
from collections import deque
from contextlib import ExitStack

import numpy as np
import ml_dtypes

import concourse.bass as bass
import concourse.tile as tile
from concourse import bacc, mybir
from concourse.bass_utils import run_bass_kernel_spmd
from concourse.masks import make_identity

F32 = mybir.dt.float32
BF16 = mybir.dt.bfloat16
AF = mybir.ActivationFunctionType
ALU = mybir.AluOpType
BF = ml_dtypes.bfloat16

# Model / sharding dims (hardcoded for this problem).
C, FDIM, H, D = 512, 2048, 8, 64
B, LQ, LM = 16, 256, 4096
NCORES = 8
BL = B // NCORES          # batch items per core
CT = C // 128             # 4 feature tiles
FT = FDIM // 128          # 16 ffn tiles
LQT = LQ // 128           # 2 query-token tiles
CHUNK = 256               # memory tokens per chunk
NCHUNK = LM // CHUNK      # 16
LKT = CHUNK // 128        # 2 key-token tiles per chunk
LN_EPS = 1e-5
SCALE = 1.0 / np.sqrt(D)  # 0.125

W_NAMES = ["sa_wq", "sa_wk", "sa_wv", "sa_wo", "ca_wq", "ca_wk", "ca_wv", "ca_wo"]


def _build_program(dbg=False):
    nc = bacc.Bacc("TRN2", target_bir_lowering=False, debug=False)

    def din(name, shape, dtype=F32):
        return nc.dram_tensor(name, list(shape), dtype, kind="ExternalInput").ap()

    q_d = din("q", (BL, LQ, C))
    mem_d = din("mem", (BL, LM, C))
    w_d = {n: din(n, (128, CT, C), BF16) for n in W_NAMES}
    w1_d = din("w1", (128, CT, FDIM), BF16)
    w2_d = din("w2", (128, FT, C), BF16)
    pq_sa_d = din("pq_sa", (128, CT, LQ), BF16)
    pk_sa_d = din("pk_sa", (128, CT, LQ), BF16)
    pq_ca_d = din("pq_ca", (128, CT, LQ), BF16)
    pk_ca_d = din("pk_ca", (128, CT, LM), BF16)
    bvp_sa_d = din("bvp_sa", (H, D + 1))
    bvp_ca_d = din("bvp_ca", (H, D + 1))
    bo_sa_d = din("bo_sa", (C,))
    bo_ca_d = din("bo_ca", (C,))
    b1_d = din("b1", (128, FT))
    b2_d = din("b2", (C,))
    ln_d = {n: din(n, (C,)) for n in ["g1", "bb1", "g2", "bb2", "g3", "bb3"]}
    out_d = nc.dram_tensor("out", [BL, LQ, C], F32, kind="ExternalOutput").ap()
    dbg_d = {}
    if dbg:
        for n, shape, dt in [
            ("dbg_qT", (128, CT, LQ), BF16),
            ("dbg_QTs", (128, CT, LQ), BF16),
            ("dbg_KTs", (128, CT, LQ), BF16),
            ("dbg_vps", (128, LQT, H, D + 1), BF16),
            ("dbg_osbsa", (D + 1, H, 256), F32),
            ("dbg_OTs", (128, CT, LQ), BF16),
            ("dbg_tgt", (128, LQT, C), F32),
            ("dbg_QTc", (128, CT, LQ), BF16),
            ("dbg_KT0", (128, CT, CHUNK), BF16),
            ("dbg_vp0", (128, LKT, H, D + 1), BF16),
            ("dbg_mT0", (128, CT, CHUNK), BF16),
            ("dbg_osbca", (D + 1, H, 256), F32),
            ("dbg_tgt2", (128, LQT, C), F32),
        ]:
            dbg_d[n] = nc.dram_tensor(n, list(shape), dt, kind="ExternalOutput").ap()

    def dump(name, t):
        if dbg:
            nc.sync.dma_start(dbg_d[name], t[:])

    def bcast(ap):
        # [..] dram AP -> partition-broadcast AP [128, ..]
        return bass.AP(tensor=ap.tensor, offset=ap.offset, ap=[[0, 128]] + list(ap.ap))

    with ExitStack() as ctx:
        tc = ctx.enter_context(tile.TileContext(nc))
        wp = ctx.enter_context(tc.tile_pool(name="wp", bufs=1))
        sbB = ctx.enter_context(tc.tile_pool(name="sbB", bufs=1))
        sbC = ctx.enter_context(tc.tile_pool(name="sbC", bufs=2))
        sbH = ctx.enter_context(tc.tile_pool(name="sbH", bufs=3))
        psw = ctx.enter_context(tc.tile_pool(name="psw", bufs=8, space="PSUM"))

        # ---- constants / weights into SBUF ----
        W = {}
        for n in W_NAMES:
            W[n] = wp.tile([128, CT, C], BF16, tag=n, name=n)
            nc.sync.dma_start(W[n][:], w_d[n])
        w1 = wp.tile([128, CT, FDIM], BF16, tag="w1", name="w1")
        nc.sync.dma_start(w1[:], w1_d)
        w2 = wp.tile([128, FT, C], BF16, tag="w2", name="w2")
        nc.sync.dma_start(w2[:], w2_d)
        pq_sa = wp.tile([128, CT, LQ], BF16, tag="pq_sa", name="pq_sa")
        nc.sync.dma_start(pq_sa[:], pq_sa_d)
        pk_sa = wp.tile([128, CT, LQ], BF16, tag="pk_sa", name="pk_sa")
        nc.sync.dma_start(pk_sa[:], pk_sa_d)
        pq_ca = wp.tile([128, CT, LQ], BF16, tag="pq_ca", name="pq_ca")
        nc.sync.dma_start(pq_ca[:], pq_ca_d)
        bvp_sa = wp.tile([128, H, D + 1], F32, tag="bvp_sa", name="bvp_sa")
        nc.sync.dma_start(bvp_sa[:], bcast(bvp_sa_d))
        bvp_ca = wp.tile([128, H, D + 1], F32, tag="bvp_ca", name="bvp_ca")
        nc.sync.dma_start(bvp_ca[:], bcast(bvp_ca_d))
        bo_sa = wp.tile([128, C], F32, tag="bo_sa", name="bo_sa")
        nc.sync.dma_start(bo_sa[:], bcast(bo_sa_d))
        bo_ca = wp.tile([128, C], F32, tag="bo_ca", name="bo_ca")
        nc.sync.dma_start(bo_ca[:], bcast(bo_ca_d))
        b2t = wp.tile([128, C], F32, tag="b2t", name="b2t")
        nc.sync.dma_start(b2t[:], bcast(b2_d))
        b1t = wp.tile([128, FT], F32, tag="b1t", name="b1t")
        nc.sync.dma_start(b1t[:], b1_d)
        ln = {}
        for n in ln_d:
            ln[n] = wp.tile([128, C], F32, tag="ln_" + n, name="ln_" + n)
            nc.sync.dma_start(ln[n][:], bcast(ln_d[n]))
        ident = wp.tile([128, 128], BF16, tag="ident", name="ident")
        make_identity(nc, ident[:])
        ones1 = wp.tile([1, D], BF16, tag="ones1", name="ones1")
        nc.vector.memset(ones1[:], 1.0)
        epst = wp.tile([128, 1], F32, tag="epst", name="epst")
        nc.vector.memset(epst[:], LN_EPS)

        # ---- filler queue: units of independent PE work interleaved into
        # dependency-stalled stretches (exp waits, LN chains).
        filler = deque()

        def fill(n):
            for _ in range(n):
                if not filler:
                    return
                filler.popleft()()

        def fill_all():
            fill(len(filler))

        # ---- helpers ----
        def transpose_unit(dst_bf, src_bf, lt):
            # one lq/lk tile: 4 PE transposes + 1 DVE evacuation
            pt = psw.tile([128, 512], BF16, tag="w", name="ptr")
            for ct in range(CT):
                nc.tensor.transpose(
                    pt[:, ct * 128:(ct + 1) * 128],
                    src_bf[:, lt, ct * 128:(ct + 1) * 128],
                    ident[:],
                )
            nc.vector.tensor_copy(
                dst_bf[:, :, lt * 128:(lt + 1) * 128],
                pt[:].rearrange("p (ct j) -> p ct j", ct=CT),
            )

        def proj_fm_unit(dst_bf, xT, w, cot, L, add_bf=None):
            # one output feature tile of a feature-major projection
            pt = psw.tile([128, 512], F32, tag="w", name="pmed")
            for kt in range(CT):
                nc.tensor.matmul(
                    pt[:, 0:L],
                    w[:, kt, cot * 128:(cot + 1) * 128],
                    xT[:, kt, :],
                    start=(kt == 0),
                    stop=(kt == CT - 1),
                )
            if add_bf is not None:
                nc.vector.tensor_add(dst_bf[:, cot, :], pt[:, 0:L], add_bf[:, cot, :])
            else:
                nc.vector.tensor_copy(dst_bf[:, cot, :], pt[:, 0:L])

        def vprime_unit(vp_bf, lkt, xT, w, bvp_t):
            # one token tile of the V projection, evacuated as V' (with ones)
            pv = psw.tile([128, 512], F32, tag="w", name="pv")
            for kt in range(CT):
                nc.tensor.matmul(
                    pv[:],
                    xT[:, kt, lkt * 128:(lkt + 1) * 128],
                    w[:, kt, :],
                    start=(kt == 0),
                    stop=(kt == CT - 1),
                )
            nc.vector.tensor_add(
                vp_bf[:, lkt, :, 0:D],
                pv[:].rearrange("p (h d) -> p h d", h=H),
                bvp_t[:, :, 0:D],
            )
            nc.vector.tensor_copy(vp_bf[:, lkt, :, D:D + 1], bvp_t[:, :, D:D + 1])

        def attn_heads(QT, KT, vp_bf, osb, first, fill_per_head=1):
            # S^T -> exp -> per-chunk O'^T in PSUM, accumulated into the SBUF
            # tile osb [D+1, H, 256].  Heads are processed in pairs: the even
            # head uses array rows 0-63 and the odd head rows 64-127, so
            # their score matmuls run concurrently in distinct row-groups.
            for hp in range(H // 2):
                scs, ess = [], []
                for j in range(2):
                    sc = psw.tile([128, 512], F32, tag="w", name="sc")
                    scs.append(sc)
                for k in range(LKT):
                    for j in range(2):
                        nc.tensor.matmul(
                            scs[j][:, k * 256:(k + 1) * 256],
                            KT[j * 64:j * 64 + 64, hp, k * 128:(k + 1) * 128],
                            QT[j * 64:j * 64 + 64, hp, :],
                            start=True,
                            stop=True,
                        )
                for j in range(2):
                    es = sbH.tile([128, LKT, 256], BF16, tag="expst", name="es",
                                  bufs=4)
                    nc.scalar.activation(
                        es[:].rearrange("p a b -> p (a b)"),
                        scs[j][:],
                        AF.Exp,
                        scale=SCALE,
                    )
                    ess.append(es)
                fill(fill_per_head)
                for j in range(2):
                    h = 2 * hp + j
                    op = psw.tile([D + 1, 256], F32, tag="w", name="op")
                    for k in range(LKT):
                        nc.tensor.matmul(
                            op[:],
                            vp_bf[:, k, h, :],
                            ess[j][:, k, :],
                            start=(k == 0),
                            stop=(k == LKT - 1),
                        )
                    if first:
                        nc.vector.tensor_copy(osb[:, h, :], op[:])
                    else:
                        nc.vector.tensor_add(osb[:, h, :], osb[:, h, :], op[:])

        def evac_normalize(osb, OT):
            # Broadcast sumexp across 64 partitions via a ones-matmul, then
            # take the reciprocal on the multi-partition tile (single-
            # partition custom-DVE reciprocal misbehaves on HW) and scale.
            for h in range(H):
                po, ct = (h % 2) * 64, h // 2
                rsb = sbH.tile([1, 256], BF16, tag="rsb", name="rsb")
                nc.vector.tensor_copy(rsb[:], osb[D:D + 1, h, :])
                bc = psw.tile([D, 256], F32, tag="w", name="bc")
                nc.tensor.matmul(bc[:], ones1[:], rsb[:], start=True, stop=True)
                bcf = sbH.tile([D, 256], F32, tag="bcf", name="bcf", bufs=2)
                nc.vector.tensor_copy(bcf[:], bc[:])
                rcp = sbH.tile([D, 256], F32, tag="rcp", name="rcp", bufs=2)
                nc.vector.reciprocal_approx_fast(rcp[:], bcf[:])
                nc.vector.tensor_mul(OT[po:po + 64, ct, :], osb[0:D, h, :], rcp[:])

        def residual_ln(dst, psum, res, bo_t, g_t, b_t):
            x = sbH.tile([128, C], F32, tag="lnx", name="lnx", bufs=2)
            nc.vector.tensor_add(x[:], psum, res)
            nc.vector.tensor_add(x[:], x[:], bo_t[:])
            st = sbH.tile([128, 6], F32, tag="st", name="st")
            nc.vector.bn_stats(st[:], x[:])
            mv = sbH.tile([128, 2], F32, tag="mv", name="mv")
            nc.vector.bn_aggr(mv[:], st[:])
            sd = sbH.tile([128, 1], F32, tag="sd", name="sd")
            nc.scalar.activation(sd[:], mv[:, 1:2], AF.Sqrt, bias=epst[:])
            rstd = sbH.tile([128, 1], F32, tag="rstd", name="rstd")
            nc.vector.reciprocal_approx_fast(rstd[:], sd[:])
            t1 = sbH.tile([128, C], F32, tag="lnt", name="lnt", bufs=2)
            nc.vector.tensor_scalar(
                t1[:], x[:], mv[:, 0:1], rstd[:], ALU.subtract, ALU.mult
            )
            nc.vector.tensor_mul(t1[:], t1[:], g_t[:])
            nc.vector.tensor_add(dst, t1[:], b_t[:])

        def out_proj(OT, w):
            pps = []
            for lt in range(LQT):
                pp = psw.tile([128, 512], F32, tag="w", name="pp")
                for ct in range(CT):
                    nc.tensor.matmul(
                        pp[:],
                        OT[:, ct, lt * 128:(lt + 1) * 128],
                        w[:, ct, :],
                        start=(ct == 0),
                        stop=(ct == CT - 1),
                    )
                pps.append(pp)
            return pps

        def push_chunk_prep(b, ci, state):
            # Emit DMA/cast now; queue the PE-heavy prep as filler units.
            mf = sbC.tile([128, LKT, C], F32, tag="mf", name="mf")
            nc.sync.dma_start(
                mf[:],
                mem_d[b, ci * CHUNK:(ci + 1) * CHUNK].rearrange(
                    "(lt p) c -> p lt c", p=128
                ),
            )
            mbf = sbC.tile([128, LKT, C], BF16, tag="mbf", name="mbf", bufs=3)
            nc.vector.tensor_copy(mbf[:], mf[:])
            pkc = sbC.tile([128, CT, CHUNK], BF16, tag="pkc", name="pkc", bufs=3)
            nc.sync.dma_start(pkc[:], pk_ca_d[:, :, ci * CHUNK:(ci + 1) * CHUNK])
            mT = sbC.tile([128, CT, CHUNK], BF16, tag="mT", name="mT", bufs=3)
            KTc = sbC.tile([128, CT, CHUNK], BF16, tag="KTc", name="KTc", bufs=5)
            vpc = sbC.tile([128, LKT, H, D + 1], BF16, tag="vpc", name="vpc", bufs=5)
            state[(b, ci)] = (KTc, vpc)
            for lkt in range(LKT):
                filler.append(lambda lkt=lkt: transpose_unit(mT, mbf, lkt))
            for cot in range(CT):
                filler.append(
                    lambda cot=cot: proj_fm_unit(KTc, mT, W["ca_wk"], cot, CHUNK,
                                                 add_bf=pkc)
                )
            for lkt in range(LKT):
                filler.append(
                    lambda lkt=lkt: vprime_unit(vpc, lkt, mT, W["ca_wv"], bvp_ca)
                )

        def q_prep(b):
            qf = sbB.tile([128, LQT, C], F32, tag="qf", name="qf")
            nc.sync.dma_start(qf[:], q_d[b].rearrange("(lt p) c -> p lt c", p=128))
            qbf = sbB.tile([128, LQT, C], BF16, tag="xbf", name="qbf", bufs=2)
            nc.vector.tensor_copy(qbf[:], qf[:])
            qT = sbB.tile([128, CT, LQ], BF16, tag="xT", name="qT", bufs=2)
            for lt in range(LQT):
                filler.append(lambda lt=lt: transpose_unit(qT, qbf, lt))
            return qf, qT

        def sa_proj(b, qT):
            QTs = sbB.tile([128, CT, LQ], BF16, tag="QT", name="QTs", bufs=2)
            KTs = sbB.tile([128, CT, LQ], BF16, tag="KTs", name="KTs")
            vps = sbB.tile([128, LQT, H, D + 1], BF16, tag="vps", name="vps")
            for cot in range(CT):
                filler.append(
                    lambda cot=cot: proj_fm_unit(QTs, qT, W["sa_wq"], cot, LQ,
                                                 add_bf=pq_sa)
                )
            for cot in range(CT):
                filler.append(
                    lambda cot=cot: proj_fm_unit(KTs, qT, W["sa_wk"], cot, LQ,
                                                 add_bf=pk_sa)
                )
            for lt in range(LQT):
                filler.append(
                    lambda lt=lt: vprime_unit(vps, lt, qT, W["sa_wv"], bvp_sa)
                )
            return QTs, KTs, vps

        # ---- main flow ----
        qprep = {}
        saprep = {}
        kv_state = {}
        qprep[0] = q_prep(0)
        saprep[0] = sa_proj(0, qprep[0][1])
        fill_all()
        push_chunk_prep(0, 0, kv_state)
        push_chunk_prep(0, 1, kv_state)

        for b in range(BL):
            qf, qT = qprep[b]
            QTs, KTs, vps = saprep[b]

            # self attention (LQ tokens play the role of one "chunk");
            # fillers consume chunk 0/1 prep queued by the previous batch's
            # LN2 section (or the preamble for b=0).
            osb_sa = sbB.tile([D + 1, H, 256], F32, tag="osb", name="osb_sa")
            attn_heads(QTs, KTs, vps, osb_sa, first=True, fill_per_head=4)
            fill_all()
            if b == 0:
                dump("dbg_qT", qT)
                dump("dbg_QTs", QTs)
                dump("dbg_KTs", KTs)
                dump("dbg_vps", vps)
                dump("dbg_osbsa", osb_sa)

            push_chunk_prep(b, 2, kv_state)
            push_chunk_prep(b, 3, kv_state)
            OTs = sbB.tile([128, CT, LQ], BF16, tag="OT", name="OTs")
            evac_normalize(osb_sa, OTs)
            pps = out_proj(OTs, W["sa_wo"])
            fill_all()
            tgt = sbB.tile([128, LQT, C], F32, tag="tgt", name="tgt")
            for lt, pp in enumerate(pps):
                residual_ln(tgt[:, lt, :], pp[:], qf[:, lt, :], bo_sa,
                            ln["g1"], ln["bb1"])
            if b == 0:
                dump("dbg_OTs", OTs)
                dump("dbg_tgt", tgt)
            tgtbf = sbB.tile([128, LQT, C], BF16, tag="xbf", name="tgtbf", bufs=2)
            nc.vector.tensor_copy(tgtbf[:], tgt[:])
            tgtT = sbB.tile([128, CT, LQ], BF16, tag="xT", name="tgtT", bufs=2)
            for lt in range(LQT):
                transpose_unit(tgtT, tgtbf, lt)
            QTc = sbB.tile([128, CT, LQ], BF16, tag="QT", name="QTc", bufs=2)
            for cot in range(CT):
                proj_fm_unit(QTc, tgtT, W["ca_wq"], cot, LQ, add_bf=pq_ca)

            if b == 0:
                dump("dbg_QTc", QTc)
                KT0, vp0 = kv_state[(0, 0)]
                dump("dbg_KT0", KT0)
                dump("dbg_vp0", vp0)
            # cross attention over memory chunks
            osb_ca = sbB.tile([D + 1, H, 256], F32, tag="osb", name="osb_ca")
            for ci in range(NCHUNK):
                if ci + 4 < NCHUNK:
                    push_chunk_prep(b, ci + 4, kv_state)
                elif b + 1 < BL:
                    if ci == NCHUNK - 4:
                        qprep[b + 1] = q_prep(b + 1)
                    elif ci == NCHUNK - 3:
                        saprep[b + 1] = sa_proj(b + 1, qprep[b + 1][1])
                KTc, vpc = kv_state.pop((b, ci))
                attn_heads(QTc, KTc, vpc, osb_ca, first=(ci == 0),
                           fill_per_head=2)

            if b == 0:
                dump("dbg_osbca", osb_ca)
            OTc = sbB.tile([128, CT, LQ], BF16, tag="OT", name="OTc")
            evac_normalize(osb_ca, OTc)
            pps = out_proj(OTc, W["ca_wo"])
            if b + 1 < BL:
                push_chunk_prep(b + 1, 0, kv_state)
                push_chunk_prep(b + 1, 1, kv_state)
            fill_all()
            tgt2 = sbB.tile([128, LQT, C], F32, tag="tgt2", name="tgt2")
            for lt, pp in enumerate(pps):
                residual_ln(tgt2[:, lt, :], pp[:], tgt[:, lt, :], bo_ca,
                            ln["g2"], ln["bb2"])

            if b == 0:
                dump("dbg_tgt2", tgt2)
            tgt2bf = sbB.tile([128, LQT, C], BF16, tag="xbf", name="tgt2bf", bufs=2)
            nc.vector.tensor_copy(tgt2bf[:], tgt2[:])
            tgt2T = sbB.tile([128, CT, LQ], BF16, tag="xT", name="tgt2T", bufs=2)
            for lt in range(LQT):
                transpose_unit(tgt2T, tgt2bf, lt)

            # ffn
            hT = sbB.tile([128, FT, LQ], BF16, tag="hT", name="hT")
            for ft in range(FT):
                ph = psw.tile([128, 512], F32, tag="w", name="ph")
                for kt in range(CT):
                    nc.tensor.matmul(
                        ph[:, 0:LQ],
                        w1[:, kt, ft * 128:(ft + 1) * 128],
                        tgt2T[:, kt, :],
                        start=(kt == 0),
                        stop=(kt == CT - 1),
                    )
                nc.vector.tensor_scalar(
                    hT[:, ft, :], ph[:, 0:LQ], b1t[:, ft:ft + 1], 0.0,
                    ALU.add, ALU.max
                )
            outf = sbB.tile([128, LQT, C], F32, tag="outf", name="outf")
            out_tm = out_d[b].rearrange("(lt p) c -> p lt c", p=128)
            for lt in range(LQT):
                pw = psw.tile([128, 512], F32, tag="w", name="pw")
                for ft in range(FT):
                    nc.tensor.matmul(
                        pw[:],
                        hT[:, ft, lt * 128:(lt + 1) * 128],
                        w2[:, ft, :],
                        start=(ft == 0),
                        stop=(ft == FT - 1),
                    )
                residual_ln(outf[:, lt, :], pw[:], tgt2[:, lt, :], b2t,
                            ln["g3"], ln["bb3"])
                nc.sync.dma_start(out_tm[:, lt, :], outf[:, lt, :])

    nc.compile()
    return nc


_PROG = None
LAST_RESULTS = None


def _get_prog():
    global _PROG
    if _PROG is None:
        _PROG = _build_program(dbg=bool(os.environ.get("KERNEL_DEBUG")))
    return _PROG


def _pe_table(L, Cc):
    pos = np.arange(L, dtype=np.float64)[:, None]
    item = 10000.0 ** (np.arange(0, Cc, 2, dtype=np.float64) / Cc)
    pe = np.zeros((L, Cc), np.float32)
    pe[:, 0::2] = np.sin(pos / item)
    pe[:, 1::2] = np.cos(pos / item)
    return pe


def _wtiles(w):
    # [Cin, Cout] -> [128, Cin//128, Cout]
    cin, cout = w.shape
    return np.ascontiguousarray(
        w.reshape(cin // 128, 128, cout).transpose(1, 0, 2)
    )


def _fm(x):
    # [L, C] -> feature-major tiled [128, C//128, L]
    xT = x.T  # [C, L]
    return np.ascontiguousarray(
        xT.reshape(xT.shape[0] // 128, 128, xT.shape[1]).transpose(1, 0, 2)
    )


def kernel(**inputs):
    global LAST_RESULTS
    nc = _get_prog()
    f = {k: np.asarray(v, dtype=np.float32) for k, v in inputs.items()}
    qpos = _pe_table(LQ, C)
    mpos = _pe_table(LM, C)

    def vprime_bias(bv):
        bvp = np.zeros((H, D + 1), np.float32)
        bvp[:, :D] = bv.reshape(H, D)
        bvp[:, D] = 1.0
        return bvp

    common = {
        "w1": _wtiles(f["ffn_w1"]).astype(BF),
        "w2": _wtiles(f["ffn_w2"]).astype(BF),
        "pq_sa": _fm(qpos @ f["sa_wq"] + f["sa_bq"]).astype(BF),
        "pk_sa": _fm(qpos @ f["sa_wk"] + f["sa_bk"]).astype(BF),
        "pq_ca": _fm(qpos @ f["ca_wq"] + f["ca_bq"]).astype(BF),
        "pk_ca": _fm(mpos @ f["ca_wk"] + f["ca_bk"]).astype(BF),
        "bvp_sa": vprime_bias(f["sa_bv"]),
        "bvp_ca": vprime_bias(f["ca_bv"]),
        "bo_sa": f["sa_bo"],
        "bo_ca": f["ca_bo"],
        "b1": np.ascontiguousarray(f["ffn_b1"].reshape(FT, 128).T),
        "b2": f["ffn_b2"],
        "g1": f["ln1_g"], "bb1": f["ln1_b"],
        "g2": f["ln2_g"], "bb2": f["ln2_b"],
        "g3": f["ln3_g"], "bb3": f["ln3_b"],
    }
    for n in W_NAMES:
        common[n] = _wtiles(f[n]).astype(BF)

    query = f["query"]
    memory = f["memory"]
    in_maps = []
    for core in range(NCORES):
        m = dict(common)
        m["q"] = np.ascontiguousarray(query[core * BL:(core + 1) * BL])
        m["mem"] = np.ascontiguousarray(memory[core * BL:(core + 1) * BL])
        in_maps.append(m)

    trace = bool(os.environ.get("KERNEL_TRACE"))
    res = run_bass_kernel_spmd(
        nc, in_maps, core_ids=list(range(NCORES)), trace=trace
    )
    LAST_RESULTS = res
    out = np.concatenate([res.results[i]["out"] for i in range(NCORES)], axis=0)
    return out.astype(np.float32)


# revision 17
# speedup vs baseline: 1.0222x; 1.0222x over previous
"""Trainium2 Bass kernel for a transformer decoder layer (self-attn +
cross-attn + FFN), data-parallel over batch across 8 NeuronCores.

Layout strategy per core (2 batch items per core):
  - Activations feeding matmuls are kept feature-major ([C, L] with C on
    partitions) so projections contract over the partition dim; LN /
    residual work is token-major ([L, C]).  Transposes go through the PE
    (bf16, via identity matmul).
  - Attention uses the S^T formulation: S^T[lk, lq] tiles come straight
    out of matmuls with K/Q feature-major; exp on ScalarE; att@V uses V
    token-major (augmented with a ones column so sumexp falls out of the
    same matmul), accumulated in a persistent PSUM tile across chunks.
  - Positional-encoding projections (qpos@Wq etc.) are precomputed on the
    host and folded in as additive terms, so `memory` only needs one
    on-device transpose.
  - All matmul operands are bf16 (fp32 accumulate in PSUM); everything
    else (softmax recip, LN, residuals) is fp32.
  - Emission is software-pipelined: memory-chunk prep work (DMA, cast,
    transpose, K/V projection) for chunk c+2 is interleaved between the
    per-head score/exp/att-V steps of chunk c via a filler queue, so the
    in-order PE queue always has independent matmuls while ScalarE's exp
    and the LN chains run.
"""

import os
from collections import deque
from contextlib import ExitStack

import numpy as np
import ml_dtypes

import concourse.bass as bass
import concourse.tile as tile
from concourse import bacc, mybir
from concourse.bass_utils import run_bass_kernel_spmd
from concourse.masks import make_identity

F32 = mybir.dt.float32
BF16 = mybir.dt.bfloat16
AF = mybir.ActivationFunctionType
ALU = mybir.AluOpType
BF = ml_dtypes.bfloat16

# Model / sharding dims (hardcoded for this problem).
C, FDIM, H, D = 512, 2048, 8, 64
B, LQ, LM = 16, 256, 4096
NCORES = 8
BL = B // NCORES          # batch items per core
CT = C // 128             # 4 feature tiles
FT = FDIM // 128          # 16 ffn tiles
LQT = LQ // 128           # 2 query-token tiles
CHUNK = 256               # memory tokens per chunk
NCHUNK = LM // CHUNK      # 16
LKT = CHUNK // 128        # 2 key-token tiles per chunk
LN_EPS = 1e-5
SCALE = 1.0 / np.sqrt(D)  # 0.125

W_NAMES = ["sa_wq", "sa_wk", "sa_wv", "sa_wo", "ca_wq", "ca_wk", "ca_wv", "ca_wo"]


def _build_program(dbg=False):
    nc = bacc.Bacc("TRN2", target_bir_lowering=False, debug=False)

    def din(name, shape, dtype=F32):
        return nc.dram_tensor(name, list(shape), dtype, kind="ExternalInput").ap()

    q_d = din("q", (BL, LQ, C))
    mem_d = din("mem", (BL, LM, C))
    w_d = {n: din(n, (128, CT, C), BF16) for n in W_NAMES}
    w1_d = din("w1", (128, CT, FDIM), BF16)
    w2_d = din("w2", (128, FT, C), BF16)
    pq_sa_d = din("pq_sa", (128, CT, LQ), BF16)
    pk_sa_d = din("pk_sa", (128, CT, LQ), BF16)
    pq_ca_d = din("pq_ca", (128, CT, LQ), BF16)
    pk_ca_d = din("pk_ca", (128, CT, LM), BF16)
    bvp_sa_d = din("bvp_sa", (H, D + 1))
    bvp_ca_d = din("bvp_ca", (H, D + 1))
    bo_sa_d = din("bo_sa", (C,))
    bo_ca_d = din("bo_ca", (C,))
    b1_d = din("b1", (128, FT))
    b2_d = din("b2", (C,))
    ln_d = {n: din(n, (C,)) for n in ["g1", "bb1", "g2", "bb2", "g3", "bb3"]}
    out_d = nc.dram_tensor("out", [BL, LQ, C], F32, kind="ExternalOutput").ap()
    dbg_d = {}
    if dbg:
        for n, shape, dt in [
            ("dbg_qT", (128, CT, LQ), BF16),
            ("dbg_QTs", (128, CT, LQ), BF16),
            ("dbg_KTs", (128, CT, LQ), BF16),
            ("dbg_vps", (128, LQT, H, D + 1), BF16),
            ("dbg_osbsa", (D + 1, H, 256), F32),
            ("dbg_OTs", (128, CT, LQ), BF16),
            ("dbg_tgt", (128, LQT, C), F32),
            ("dbg_QTc", (128, CT, LQ), BF16),
            ("dbg_KT0", (128, CT, CHUNK), BF16),
            ("dbg_vp0", (128, LKT, H, D + 1), BF16),
            ("dbg_mT0", (128, CT, CHUNK), BF16),
            ("dbg_osbca", (D + 1, H, 256), F32),
            ("dbg_tgt2", (128, LQT, C), F32),
        ]:
            dbg_d[n] = nc.dram_tensor(n, list(shape), dt, kind="ExternalOutput").ap()

    def dump(name, t):
        if dbg:
            nc.sync.dma_start(dbg_d[name], t[:])

    def bcast(ap):
        # [..] dram AP -> partition-broadcast AP [128, ..]
        return bass.AP(tensor=ap.tensor, offset=ap.offset, ap=[[0, 128]] + list(ap.ap))

    with ExitStack() as ctx:
        tc = ctx.enter_context(tile.TileContext(nc))
        wp = ctx.enter_context(tc.tile_pool(name="wp", bufs=1))
        sbB = ctx.enter_context(tc.tile_pool(name="sbB", bufs=1))
        sbC = ctx.enter_context(tc.tile_pool(name="sbC", bufs=2))
        sbH = ctx.enter_context(tc.tile_pool(name="sbH", bufs=3))
        psw = ctx.enter_context(tc.tile_pool(name="psw", bufs=4, space="PSUM"))
        pso = ctx.enter_context(tc.tile_pool(name="pso", bufs=1, space="PSUM"))

        # ---- constants / weights into SBUF ----
        W = {}
        for n in W_NAMES:
            W[n] = wp.tile([128, CT, C], BF16, tag=n, name=n)
            nc.sync.dma_start(W[n][:], w_d[n])
        w1 = wp.tile([128, CT, FDIM], BF16, tag="w1", name="w1")
        nc.sync.dma_start(w1[:], w1_d)
        w2 = wp.tile([128, FT, C], BF16, tag="w2", name="w2")
        nc.sync.dma_start(w2[:], w2_d)
        pq_sa = wp.tile([128, CT, LQ], BF16, tag="pq_sa", name="pq_sa")
        nc.sync.dma_start(pq_sa[:], pq_sa_d)
        pk_sa = wp.tile([128, CT, LQ], BF16, tag="pk_sa", name="pk_sa")
        nc.sync.dma_start(pk_sa[:], pk_sa_d)
        pq_ca = wp.tile([128, CT, LQ], BF16, tag="pq_ca", name="pq_ca")
        nc.sync.dma_start(pq_ca[:], pq_ca_d)
        bvp_sa = wp.tile([128, H, D + 1], F32, tag="bvp_sa", name="bvp_sa")
        nc.sync.dma_start(bvp_sa[:], bcast(bvp_sa_d))
        bvp_ca = wp.tile([128, H, D + 1], F32, tag="bvp_ca", name="bvp_ca")
        nc.sync.dma_start(bvp_ca[:], bcast(bvp_ca_d))
        bo_sa = wp.tile([128, C], F32, tag="bo_sa", name="bo_sa")
        nc.sync.dma_start(bo_sa[:], bcast(bo_sa_d))
        bo_ca = wp.tile([128, C], F32, tag="bo_ca", name="bo_ca")
        nc.sync.dma_start(bo_ca[:], bcast(bo_ca_d))
        b2t = wp.tile([128, C], F32, tag="b2t", name="b2t")
        nc.sync.dma_start(b2t[:], bcast(b2_d))
        b1t = wp.tile([128, FT], F32, tag="b1t", name="b1t")
        nc.sync.dma_start(b1t[:], b1_d)
        ln = {}
        for n in ln_d:
            ln[n] = wp.tile([128, C], F32, tag="ln_" + n, name="ln_" + n)
            nc.sync.dma_start(ln[n][:], bcast(ln_d[n]))
        ident = wp.tile([128, 128], BF16, tag="ident", name="ident")
        make_identity(nc, ident[:])
        ones1 = wp.tile([1, D], BF16, tag="ones1", name="ones1")
        nc.vector.memset(ones1[:], 1.0)
        epst = wp.tile([128, 1], F32, tag="epst", name="epst")
        nc.vector.memset(epst[:], LN_EPS)

        # ---- filler queue: units of independent PE work interleaved into
        # dependency-stalled stretches (exp waits, LN chains).
        filler = deque()

        def fill(n):
            for _ in range(n):
                if not filler:
                    return
                filler.popleft()()

        def fill_all():
            fill(len(filler))

        # ---- helpers ----
        def transpose_unit(dst_bf, src_bf, lt):
            # one lq/lk tile: 4 PE transposes + 1 DVE evacuation
            pt = psw.tile([128, 512], BF16, tag="w", name="ptr")
            for ct in range(CT):
                nc.tensor.transpose(
                    pt[:, ct * 128:(ct + 1) * 128],
                    src_bf[:, lt, ct * 128:(ct + 1) * 128],
                    ident[:],
                )
            nc.scalar.copy(
                dst_bf[:, :, lt * 128:(lt + 1) * 128],
                pt[:].rearrange("p (ct j) -> p ct j", ct=CT),
            )

        def proj_fm_unit(dst_bf, xT, w, cot, L, add_bf=None):
            # one output feature tile of a feature-major projection
            pt = psw.tile([128, 512], F32, tag="w", name="pmed")
            for kt in range(CT):
                nc.tensor.matmul(
                    pt[:, 0:L],
                    w[:, kt, cot * 128:(cot + 1) * 128],
                    xT[:, kt, :],
                    start=(kt == 0),
                    stop=(kt == CT - 1),
                )
            if add_bf is not None:
                nc.vector.tensor_add(dst_bf[:, cot, :], pt[:, 0:L], add_bf[:, cot, :])
            else:
                nc.vector.tensor_copy(dst_bf[:, cot, :], pt[:, 0:L])

        def vprime_unit(vp_bf, lkt, xT, w, bvp_t):
            # one token tile of the V projection, evacuated as V' (with ones)
            pv = psw.tile([128, 512], F32, tag="w", name="pv")
            for kt in range(CT):
                nc.tensor.matmul(
                    pv[:],
                    xT[:, kt, lkt * 128:(lkt + 1) * 128],
                    w[:, kt, :],
                    start=(kt == 0),
                    stop=(kt == CT - 1),
                )
            nc.vector.tensor_add(
                vp_bf[:, lkt, :, 0:D],
                pv[:].rearrange("p (h d) -> p h d", h=H),
                bvp_t[:, :, 0:D],
            )
            nc.vector.tensor_copy(vp_bf[:, lkt, :, D:D + 1], bvp_t[:, :, D:D + 1])

        def attn_heads(QT, KT, vp_bf, oacc, first, last, fill_per_head=1):
            # S^T -> exp -> O'^T accumulated in PSUM across all chunks
            # (oacc [D+1, H, 256]).  Heads are processed in pairs: the even
            # head uses array rows 0-63 and the odd head rows 64-127, so
            # their score matmuls run concurrently in distinct row-groups.
            # PSUM accumulation groups are per *bank* (= head pair): start
            # clears the whole bank, so only the even head's first matmul
            # starts the group and only the odd head's last matmul stops it.
            for hp in range(H // 2):
                scs, ess = [], []
                for j in range(2):
                    sc = psw.tile([128, 512], F32, tag="w", name="sc")
                    scs.append(sc)
                for k in range(LKT):
                    for j in range(2):
                        nc.tensor.matmul(
                            scs[j][:, k * 256:(k + 1) * 256],
                            KT[j * 64:j * 64 + 64, hp, k * 128:(k + 1) * 128],
                            QT[j * 64:j * 64 + 64, hp, :],
                            start=True,
                            stop=True,
                        )
                for j in range(2):
                    es = sbH.tile([128, LKT, 256], BF16, tag="expst", name="es",
                                  bufs=3)
                    nc.scalar.activation(
                        es[:].rearrange("p a b -> p (a b)"),
                        scs[j][:],
                        AF.Exp,
                        scale=SCALE,
                    )
                    ess.append(es)
                fill(fill_per_head)
                for j in range(2):
                    h = 2 * hp + j
                    for k in range(LKT):
                        nc.tensor.matmul(
                            oacc[:, h, :],
                            vp_bf[:, k, h, :],
                            ess[j][:, k, :],
                            start=(first and j == 0 and k == 0),
                            stop=(last and j == 1 and k == LKT - 1),
                        )

        def evac_normalize(oacc, OT):
            # Evacuate the PSUM accumulator once, then normalize.
            osb = sbB.tile([D + 1, H, 256], F32, tag="osb", name="osb")
            nc.vector.tensor_copy(osb[:], oacc[:])
            # Broadcast sumexp across 64 partitions via a ones-matmul, then
            # take the reciprocal on the multi-partition tile (single-
            # partition custom-DVE reciprocal misbehaves on HW) and scale.
            for h in range(H):
                po, ct = (h % 2) * 64, h // 2
                rsb = sbH.tile([1, 256], BF16, tag="rsb", name="rsb")
                nc.vector.tensor_copy(rsb[:], osb[D:D + 1, h, :])
                bc = psw.tile([D, 256], F32, tag="w", name="bc")
                nc.tensor.matmul(bc[:], ones1[:], rsb[:], start=True, stop=True)
                bcf = sbH.tile([D, 256], F32, tag="bcf", name="bcf", bufs=2)
                nc.vector.tensor_copy(bcf[:], bc[:])
                nc.vector.reciprocal_approx_fast(bcf[:], bcf[:])
                nc.vector.tensor_mul(OT[po:po + 64, ct, :], osb[0:D, h, :], bcf[:])

        def residual_ln(dst, psum, res, bo_t, g_t, b_t):
            x = sbH.tile([128, C], F32, tag="lnx", name="lnx", bufs=2)
            nc.vector.tensor_add(x[:], psum, res)
            nc.vector.tensor_add(x[:], x[:], bo_t[:])
            st = sbH.tile([128, 6], F32, tag="st", name="st")
            nc.vector.bn_stats(st[:], x[:])
            mv = sbH.tile([128, 2], F32, tag="mv", name="mv")
            nc.vector.bn_aggr(mv[:], st[:])
            sd = sbH.tile([128, 1], F32, tag="sd", name="sd")
            nc.scalar.activation(sd[:], mv[:, 1:2], AF.Sqrt, bias=epst[:])
            rstd = sbH.tile([128, 1], F32, tag="rstd", name="rstd")
            nc.vector.reciprocal_approx_fast(rstd[:], sd[:])
            t1 = sbH.tile([128, C], F32, tag="lnt", name="lnt", bufs=2)
            nc.vector.tensor_scalar(
                t1[:], x[:], mv[:, 0:1], rstd[:], ALU.subtract, ALU.mult
            )
            nc.vector.tensor_mul(t1[:], t1[:], g_t[:])
            nc.vector.tensor_add(dst, t1[:], b_t[:])

        def out_proj(OT, w):
            pps = []
            for lt in range(LQT):
                pp = psw.tile([128, 512], F32, tag="w", name="pp")
                for ct in range(CT):
                    nc.tensor.matmul(
                        pp[:],
                        OT[:, ct, lt * 128:(lt + 1) * 128],
                        w[:, ct, :],
                        start=(ct == 0),
                        stop=(ct == CT - 1),
                    )
                pps.append(pp)
            return pps

        def push_chunk_prep(b, ci, state):
            # Emit DMA/cast now; queue the PE-heavy prep as filler units.
            mf = sbC.tile([128, LKT, C], F32, tag="mf", name="mf")
            nc.sync.dma_start(
                mf[:],
                mem_d[b, ci * CHUNK:(ci + 1) * CHUNK].rearrange(
                    "(lt p) c -> p lt c", p=128
                ),
            )
            mbf = sbC.tile([128, LKT, C], BF16, tag="mbf", name="mbf", bufs=3)
            nc.vector.tensor_copy(mbf[:], mf[:])
            pkc = sbC.tile([128, CT, CHUNK], BF16, tag="pkc", name="pkc", bufs=3)
            nc.sync.dma_start(pkc[:], pk_ca_d[:, :, ci * CHUNK:(ci + 1) * CHUNK])
            mT = sbC.tile([128, CT, CHUNK], BF16, tag="mT", name="mT", bufs=3)
            KTc = sbC.tile([128, CT, CHUNK], BF16, tag="KTc", name="KTc", bufs=6)
            vpc = sbC.tile([128, LKT, H, D + 1], BF16, tag="vpc", name="vpc", bufs=6)
            state[(b, ci)] = (KTc, vpc)
            for lkt in range(LKT):
                filler.append(lambda lkt=lkt: transpose_unit(mT, mbf, lkt))
            for cot in range(CT):
                filler.append(
                    lambda cot=cot: proj_fm_unit(KTc, mT, W["ca_wk"], cot, CHUNK,
                                                 add_bf=pkc)
                )
            for lkt in range(LKT):
                filler.append(
                    lambda lkt=lkt: vprime_unit(vpc, lkt, mT, W["ca_wv"], bvp_ca)
                )

        def q_prep(b):
            qf = sbB.tile([128, LQT, C], F32, tag="qf", name="qf")
            nc.sync.dma_start(qf[:], q_d[b].rearrange("(lt p) c -> p lt c", p=128))
            qbf = sbB.tile([128, LQT, C], BF16, tag="xbf", name="qbf", bufs=2)
            nc.vector.tensor_copy(qbf[:], qf[:])
            qT = sbB.tile([128, CT, LQ], BF16, tag="xT", name="qT", bufs=2)
            for lt in range(LQT):
                filler.append(lambda lt=lt: transpose_unit(qT, qbf, lt))
            return qf, qT

        def sa_proj(b, qT):
            QTs = sbB.tile([128, CT, LQ], BF16, tag="QT", name="QTs", bufs=2)
            KTs = sbB.tile([128, CT, LQ], BF16, tag="KTs", name="KTs")
            vps = sbB.tile([128, LQT, H, D + 1], BF16, tag="vps", name="vps")
            for cot in range(CT):
                filler.append(
                    lambda cot=cot: proj_fm_unit(QTs, qT, W["sa_wq"], cot, LQ,
                                                 add_bf=pq_sa)
                )
            for cot in range(CT):
                filler.append(
                    lambda cot=cot: proj_fm_unit(KTs, qT, W["sa_wk"], cot, LQ,
                                                 add_bf=pk_sa)
                )
            for lt in range(LQT):
                filler.append(
                    lambda lt=lt: vprime_unit(vps, lt, qT, W["sa_wv"], bvp_sa)
                )
            return QTs, KTs, vps

        # ---- main flow ----
        qprep = {}
        saprep = {}
        kv_state = {}
        qprep[0] = q_prep(0)
        saprep[0] = sa_proj(0, qprep[0][1])
        fill_all()
        push_chunk_prep(0, 0, kv_state)
        push_chunk_prep(0, 1, kv_state)

        for b in range(BL):
            qf, qT = qprep[b]
            QTs, KTs, vps = saprep[b]

            # self attention (LQ tokens play the role of one "chunk");
            # fillers consume chunk 0/1 prep queued by the previous batch's
            # LN2 section (or the preamble for b=0).
            oacc_sa = pso.tile([D + 1, H, 256], F32, tag="oacc", name="oacc_sa")
            attn_heads(QTs, KTs, vps, oacc_sa, first=True, last=True,
                       fill_per_head=4)
            fill_all()
            if b == 0:
                dump("dbg_qT", qT)
                dump("dbg_QTs", QTs)
                dump("dbg_KTs", KTs)
                dump("dbg_vps", vps)


            push_chunk_prep(b, 2, kv_state)
            push_chunk_prep(b, 3, kv_state)
            push_chunk_prep(b, 4, kv_state)
            OTs = sbB.tile([128, CT, LQ], BF16, tag="OT", name="OTs")
            evac_normalize(oacc_sa, OTs)
            pps = out_proj(OTs, W["sa_wo"])
            fill_all()
            tgt = sbB.tile([128, LQT, C], F32, tag="tgt", name="tgt")
            for lt, pp in enumerate(pps):
                residual_ln(tgt[:, lt, :], pp[:], qf[:, lt, :], bo_sa,
                            ln["g1"], ln["bb1"])
            if b == 0:
                dump("dbg_OTs", OTs)
                dump("dbg_tgt", tgt)
            tgtbf = sbB.tile([128, LQT, C], BF16, tag="xbf", name="tgtbf", bufs=2)
            nc.vector.tensor_copy(tgtbf[:], tgt[:])
            tgtT = sbB.tile([128, CT, LQ], BF16, tag="xT", name="tgtT", bufs=2)
            for lt in range(LQT):
                transpose_unit(tgtT, tgtbf, lt)
            QTc = sbB.tile([128, CT, LQ], BF16, tag="QT", name="QTc", bufs=2)
            for cot in range(CT):
                proj_fm_unit(QTc, tgtT, W["ca_wq"], cot, LQ, add_bf=pq_ca)

            if b == 0:
                dump("dbg_QTc", QTc)
                KT0, vp0 = kv_state[(0, 0)]
                dump("dbg_KT0", KT0)
                dump("dbg_vp0", vp0)
            # cross attention over memory chunks
            oacc_ca = pso.tile([D + 1, H, 256], F32, tag="oacc", name="oacc_ca")
            for ci in range(NCHUNK):
                if ci + 5 < NCHUNK:
                    push_chunk_prep(b, ci + 5, kv_state)
                elif b + 1 < BL:
                    if ci == NCHUNK - 5:
                        qprep[b + 1] = q_prep(b + 1)
                    elif ci == NCHUNK - 4:
                        saprep[b + 1] = sa_proj(b + 1, qprep[b + 1][1])
                KTc, vpc = kv_state.pop((b, ci))
                attn_heads(QTc, KTc, vpc, oacc_ca,
                           first=(ci == 0), last=(ci == NCHUNK - 1),
                           fill_per_head=2)


            OTc = sbB.tile([128, CT, LQ], BF16, tag="OT", name="OTc")
            evac_normalize(oacc_ca, OTc)
            pps = out_proj(OTc, W["ca_wo"])
            if b + 1 < BL:
                push_chunk_prep(b + 1, 0, kv_state)
                push_chunk_prep(b + 1, 1, kv_state)
            fill_all()
            tgt2 = sbB.tile([128, LQT, C], F32, tag="tgt2", name="tgt2")
            for lt, pp in enumerate(pps):
                residual_ln(tgt2[:, lt, :], pp[:], tgt[:, lt, :], bo_ca,
                            ln["g2"], ln["bb2"])

            if b == 0:
                dump("dbg_tgt2", tgt2)
            tgt2bf = sbB.tile([128, LQT, C], BF16, tag="xbf", name="tgt2bf", bufs=2)
            nc.vector.tensor_copy(tgt2bf[:], tgt2[:])
            tgt2T = sbB.tile([128, CT, LQ], BF16, tag="xT", name="tgt2T", bufs=2)
            for lt in range(LQT):
                transpose_unit(tgt2T, tgt2bf, lt)

            # ffn
            hT = sbB.tile([128, FT, LQ], BF16, tag="hT", name="hT")
            for ft in range(FT):
                ph = psw.tile([128, 512], F32, tag="w", name="ph")
                for kt in range(CT):
                    nc.tensor.matmul(
                        ph[:, 0:LQ],
                        w1[:, kt, ft * 128:(ft + 1) * 128],
                        tgt2T[:, kt, :],
                        start=(kt == 0),
                        stop=(kt == CT - 1),
                    )
                nc.vector.tensor_scalar(
                    hT[:, ft, :], ph[:, 0:LQ], b1t[:, ft:ft + 1], 0.0,
                    ALU.add, ALU.max
                )
            outf = sbB.tile([128, LQT, C], F32, tag="outf", name="outf")
            out_tm = out_d[b].rearrange("(lt p) c -> p lt c", p=128)
            for lt in range(LQT):
                pw = psw.tile([128, 512], F32, tag="w", name="pw")
                for ft in range(FT):
                    nc.tensor.matmul(
                        pw[:],
                        hT[:, ft, lt * 128:(lt + 1) * 128],
                        w2[:, ft, :],
                        start=(ft == 0),
                        stop=(ft == FT - 1),
                    )
                residual_ln(outf[:, lt, :], pw[:], tgt2[:, lt, :], b2t,
                            ln["g3"], ln["bb3"])
                nc.sync.dma_start(out_tm[:, lt, :], outf[:, lt, :])

    nc.compile()
    return nc


_PROG = None
LAST_RESULTS = None


def _get_prog():
    global _PROG
    if _PROG is None:
        _PROG = _build_program(dbg=bool(os.environ.get("KERNEL_DEBUG")))
    return _PROG


def _pe_table(L, Cc):
    pos = np.arange(L, dtype=np.float64)[:, None]
    item = 10000.0 ** (np.arange(0, Cc, 2, dtype=np.float64) / Cc)
    pe = np.zeros((L, Cc), np.float32)
    pe[:, 0::2] = np.sin(pos / item)
    pe[:, 1::2] = np.cos(pos / item)
    return pe


def _wtiles(w):
    # [Cin, Cout] -> [128, Cin//128, Cout]
    cin, cout = w.shape
    return np.ascontiguousarray(
        w.reshape(cin // 128, 128, cout).transpose(1, 0, 2)
    )


def _fm(x):
    # [L, C] -> feature-major tiled [128, C//128, L]
    xT = x.T  # [C, L]
    return np.ascontiguousarray(
        xT.reshape(xT.shape[0] // 128, 128, xT.shape[1]).transpose(1, 0, 2)
    )


def kernel(**inputs):
    global LAST_RESULTS
    nc = _get_prog()
    f = {k: np.asarray(v, dtype=np.float32) for k, v in inputs.items()}
    qpos = _pe_table(LQ, C)
    mpos = _pe_table(LM, C)

    def vprime_bias(bv):
        bvp = np.zeros((H, D + 1), np.float32)
        bvp[:, :D] = bv.reshape(H, D)
        bvp[:, D] = 1.0
        return bvp

    common = {
        "w1": _wtiles(f["ffn_w1"]).astype(BF),
        "w2": _wtiles(f["ffn_w2"]).astype(BF),
        "pq_sa": _fm(qpos @ f["sa_wq"] + f["sa_bq"]).astype(BF),
        "pk_sa": _fm(qpos @ f["sa_wk"] + f["sa_bk"]).astype(BF),
        "pq_ca": _fm(qpos @ f["ca_wq"] + f["ca_bq"]).astype(BF),
        "pk_ca": _fm(mpos @ f["ca_wk"] + f["ca_bk"]).astype(BF),
        "bvp_sa": vprime_bias(f["sa_bv"]),
        "bvp_ca": vprime_bias(f["ca_bv"]),
        "bo_sa": f["sa_bo"],
        "bo_ca": f["ca_bo"],
        "b1": np.ascontiguousarray(f["ffn_b1"].reshape(FT, 128).T),
        "b2": f["ffn_b2"],
        "g1": f["ln1_g"], "bb1": f["ln1_b"],
        "g2": f["ln2_g"], "bb2": f["ln2_b"],
        "g3": f["ln3_g"], "bb3": f["ln3_b"],
    }
    for n in W_NAMES:
        common[n] = _wtiles(f[n]).astype(BF)

    query = f["query"]
    memory = f["memory"]
    in_maps = []
    for core in range(NCORES):
        m = dict(common)
        m["q"] = np.ascontiguousarray(query[core * BL:(core + 1) * BL])
        m["mem"] = np.ascontiguousarray(memory[core * BL:(core + 1) * BL])
        in_maps.append(m)

    trace = bool(os.environ.get("KERNEL_TRACE"))
    res = run_bass_kernel_spmd(
        nc, in_maps, core_ids=list(range(NCORES)), trace=trace
    )
    LAST_RESULTS = res
    out = np.concatenate([res.results[i]["out"] for i in range(NCORES)], axis=0)
    return out.astype(np.float32)
